# revision 15
# baseline (speedup 1.0000x reference)
"""Trainium2 Bass kernel for EdgeSelectionRL (gnn_message_passing).

Reference math (per batch b):
    a = xa @ Wa.T            (C, H)
    c = xa @ Wb.T            (C, H)
    logit[i, j] = sum_h w2[h] * relu(a[i, h] + c[j, h] + b1[h]) + b2
    out = sigmoid(logit)     (C, C)

Sharding: pure data-parallel over batch B=8 -> one batch element per core.

Structure (per core):
  * h-permutation pairing: h-indices paired (h1[p], h2[p]) with matching w2
    signs so alpha_p = w2[h2]/w2[h1] > 0; chunk-1 rows of W1/b1 are
    pre-scaled by alpha on the host, so BOTH h-chunk matmuls share ONE
    stationary w2[h1[p]] tile.
  * producers: relu(a_i + c_j + b1) tiles [128h, 256j] made on VectorE
    (tensor_scalar add+max from SBUF bf16, 2x mode, ~196ns) and ScalarE
    (activation Relu from PSUM, ~360ns), split ~360/152.
  * per 8-pair sweep TensorE accumulates 16 N=512 matmuls into 2 PSUM banks
    at 4 col-group positions; one ScalarE sigmoid per sweep; DMA pulls the
    16 valid rows out.
"""

import numpy as np

B, C, F, H = 8, 256, 128, 256
NCORES = 8
NPAIR = C // 2            # 128 i-pairs per core
PAIRS_PER_SWEEP = 8       # 2 banks x 4 col-groups
NSWEEP = 16               # 16 sweeps of 8 pairs
SIG_DEFER_AT = 7          # emit sweep s-1's sigmoid after this pair of sweep s
NS_UNITS = 146            # of 512 producer units on ScalarE (rest VectorE)

_cached = {}


def _unit_engines():
    """Pick NS_UNITS producer units for ScalarE, spread evenly over pairs and
    ordered so consecutive ScalarE ops strictly alternate the cTp chunk m
    (PSUM bank) — same-bank back-to-back reads cost +109ns each."""
    eng = ["V"] * 512
    last_m = 1   # next S-op should use m = 1 - last_m
    debt = 0
    scount = 0
    for q in range(128):
        want = (q + 1) * NS_UNITS // 128 - q * NS_UNITS // 128 + debt
        debt = 0
        if want >= 2 and last_m == 0:
            # emission within a pair is m0-then-m1; entering with last_m==0
            # would put m0 after m0 -> take one unit (m1) and defer the rest
            debt = want - 1
            want = 1
            ms = [1]
        elif want >= 2:
            want = min(want, 2)
            debt += max(0, want - 2)
            ms = [0, 1]
        elif want == 1:
            ms = [1 - last_m]
        else:
            ms = []
        for m in ms:
            hh = (scount // 2) % 2
            eng[4 * q + 2 * m + hh] = "S"
            last_m = m
            scount += 1
    return eng


def _build():
    import concourse.bass as bass
    import concourse.bacc as bacc
    import concourse.mybir as mybir
    from concourse import tile

    fp32 = mybir.dt.float32
    bf16 = mybir.dt.bfloat16
    Alu = mybir.AluOpType
    Act = mybir.ActivationFunctionType

    engines = _unit_engines()

    nc = bacc.Bacc(None, target_bir_lowering=False)

    xat_d = nc.dram_tensor("xat", [F, C], bf16, kind="ExternalInput")
    w1t_d = nc.dram_tensor("w1t", [2 * F, H], bf16, kind="ExternalInput")
    bcv_d = nc.dram_tensor("bcv", [128, 3], fp32, kind="ExternalInput")
    w2s_d = nc.dram_tensor("w2s", [128, 64], bf16, kind="ExternalInput")
    out_d = nc.dram_tensor("out", [C, C], fp32, kind="ExternalOutput")

    with tile.TileContext(nc) as tc:
        with (
            tc.tile_pool(name="const", bufs=1) as const_pool,
            tc.tile_pool(name="rtiles", bufs=24) as r_pool,
            tc.tile_pool(name="sig", bufs=4) as sig_pool,
            tc.tile_pool(name="psum", bufs=3, space=bass.MemorySpace.PSUM) as ps_pool,
            tc.tile_pool(name="psumc", bufs=1, space=bass.MemorySpace.PSUM) as psc_pool,
        ):
            # ---- load inputs (c-part of W1 first: cT gates VectorE ramp) ----
            xat = const_pool.tile([F, C], bf16, tag="xat")
            w1t = const_pool.tile([128, 2 * H], bf16, tag="w1t")  # [p, m2*H+h]
            bcv = const_pool.tile([128, 3], fp32, tag="bcv")      # b1 c0, c1, b2
            w2s = const_pool.tile([128, 64], bf16, tag="w2s")
            nc.sync.dma_start(xat[:], xat_d[:])
            nc.sync.dma_start(w1t[:, H:2 * H], w1t_d[128:256, :])   # Wb' part
            nc.sync.dma_start(w1t[:, 0:H], w1t_d[0:128, :])         # Wa' part
            nc.sync.dma_start(bcv[:], bcv_d[:])
            nc.sync.dma_start(w2s[:], w2s_d[:])
            w1t0 = w1t[:, 0:H]
            w1t1 = w1t[:, H:2 * H]
            b1p = bcv[:, 0:2]
            b2v = bcv[:, 2:3]

            # preload the sigmoid ACT table while DMAs are in flight
            warm = const_pool.tile([128, 1], fp32, tag="warm")
            nc.scalar.activation(
                warm[:], nc.const_aps.aps[(fp32, 0.0)], Act.Sigmoid,
            )

            # ---- setup ----
            aT = [const_pool.tile([128, C], fp32, tag=f"aT{m}", name=f"aT{m}")
                  for m in range(2)]
            aTb = [const_pool.tile([128, C], fp32, tag=f"aTb{m}", name=f"aTb{m}")
                   for m in range(2)]
            cT = [const_pool.tile([128, C], bf16, tag=f"cT{m}", name=f"cT{m}")
                  for m in range(2)]
            cTp = [psc_pool.tile([128, C], fp32, tag=f"cTp{m}", name=f"cTp{m}")
                   for m in range(2)]
            aps = ps_pool.tile([128, 1024], fp32, tag="ps")
            for m in range(2):
                # c chunk m -> cTp (PSUM, stays resident for ScalarE units)
                nc.tensor.matmul(
                    cTp[m][:], w1t1[:, m * 128:(m + 1) * 128],
                    xat[:], start=True, stop=True,
                )
                # cT (bf16, +b1) for VectorE units
                nc.vector.tensor_scalar(
                    cT[m][:], cTp[m][:], b1p[:, m:m + 1], None,
                    Alu.add,
                )
                nc.tensor.matmul(
                    aps[:, m * C:(m + 1) * C], w1t0[:, m * 128:(m + 1) * 128],
                    xat[:], start=True, stop=True,
                )
                nc.vector.tensor_copy(aT[m][:], aps[:, m * C:(m + 1) * C])
                nc.scalar.activation(
                    aTb[m][:], aps[:, m * C:(m + 1) * C], Act.Identity,
                    bias=b1p[:, m:m + 1],
                )

            # ---- main loop ----
            def _emit_sig(s, ps):
                sig = sig_pool.tile([128, 1024], fp32, tag="sig", name=f"sig{s}")
                nc.scalar.activation(sig[:], ps[:], Act.Sigmoid, bias=b2v[:, 0:1])
                # valid rows: partition 32*grp, free bank*512+hh*256 ->
                # out row i = 16*s + 8*bank + 2*grp + hh
                srcap = sig[0:128:32, :].rearrange("g (b e) -> g b e", b=2)
                dstap = out_d[16 * s:16 * s + 16, :].rearrange(
                    "(b g two) j -> g b (two j)", b=2, g=4)
                nc.sync.dma_start(dstap, srcap)

            def _emit_sig_bank(bk, ps):
                # tail sweep: per-bank sigmoid, rows 240+8*bk..247+8*bk
                sigb = sig_pool.tile([128, 512], fp32, tag="sig", name=f"sigb{bk}")
                nc.scalar.activation(sigb[:], ps[:, bk * 512:(bk + 1) * 512],
                                     Act.Sigmoid, bias=b2v[:, 0:1])
                dstb = out_d[240 + 8 * bk:248 + 8 * bk, :].rearrange(
                    "(g two) j -> g (two j)", g=4)
                nc.sync.dma_start(dstb, sigb[0:128:32, :])

            def _emit_unit(q, m, hh, dst):
                u = 4 * q + 2 * m + hh
                i = 2 * q + hh
                if engines[u] == "S":
                    nc.scalar.activation(
                        dst, cTp[m][:], Act.Relu,
                        bias=aTb[m][:, i:i + 1],
                    )
                else:
                    nc.vector.tensor_scalar(
                        dst, cT[m][:], aT[m][:, i:i + 1], 0.0,
                        Alu.add, Alu.max,
                    )

            pending = None
            for s in range(NSWEEP):
                ps = ps_pool.tile([128, 1024], fp32, tag="ps")
                for t in range(PAIRS_PER_SWEEP):
                    q = s * PAIRS_PER_SWEEP + t   # pair; i = 2q, 2q+1
                    bank = t // 4
                    grp = t % 4
                    rts = [r_pool.tile([128, 512], bf16, tag="r", name=f"r{q}_{m}")
                           for m in range(2)]
                    if t == SIG_DEFER_AT and pending is not None:
                        _emit_sig(*pending)
                        pending = None
                    for m in range(2):
                        for hh in range(2):
                            _emit_unit(q, m, hh, rts[m][:, hh * 256:(hh + 1) * 256])
                    po = ps[32 * grp:32 * grp + 32, bank * 512:(bank + 1) * 512]
                    nc.tensor.matmul(po, w2s[:, 0:32], rts[0][:],
                                     start=True, stop=False,
                                     tile_position=(0, 32 * grp))
                    nc.tensor.matmul(po, w2s[:, 32:64], rts[1][:],
                                     start=False, stop=True,
                                     tile_position=(0, 32 * grp))
                    if s == NSWEEP - 1 and t == 3:
                        _emit_sig_bank(0, ps)
                pending = (s, ps)
            _emit_sig_bank(1, pending[1])

    nc.compile()
    return nc


def _prep_in_maps(xa, W1, b1, w2, b2):
    import ml_dtypes

    xa = np.asarray(xa, dtype=np.float32)
    W1 = np.asarray(W1, dtype=np.float32)
    b1 = np.asarray(b1, dtype=np.float32).reshape(H)
    w2 = np.asarray(w2, dtype=np.float32).reshape(H)
    b2 = np.float32(np.asarray(b2).reshape(()))

    # pair h indices by matching w2 sign, sorted by |w2| so alpha ~ 1.
    # slot p gets (h1[p], h2[p]); chunk-1 rows are scaled by
    # alpha_p = w2[h2[p]] / w2[h1[p]] > 0, so both chunks share the
    # stationary w2[h1[p]].
    pos = np.where(w2 > 0)[0]
    neg = np.where(w2 <= 0)[0]
    pos = pos[np.argsort(-np.abs(w2[pos]), kind="stable")]
    neg = neg[np.argsort(-np.abs(w2[neg]), kind="stable")]
    assert len(pos) % 2 == 0 and len(neg) % 2 == 0, (len(pos), len(neg))
    h1 = np.concatenate([pos[0::2], neg[0::2]])
    h2 = np.concatenate([pos[1::2], neg[1::2]])
    alpha = w2[h2] / w2[h1]                      # > 0, ~1
    perm = np.concatenate([h1, h2])              # new h-order, chunk-major
    scale = np.concatenate([np.ones(128, np.float32), alpha.astype(np.float32)])

    W1n = W1[perm] * scale[:, None]              # (H, 2F) rows permuted+scaled
    b1n = b1[perm] * scale
    w2s_col = w2[h1].astype(np.float32)          # shared stationary column

    w1t = np.ascontiguousarray(W1n.T).astype(ml_dtypes.bfloat16)  # (2F, H)
    bcv = np.empty((128, 3), dtype=np.float32)
    bcv[:, 0:2] = b1n.reshape(2, 128).T
    bcv[:, 2] = b2
    w2s = np.repeat(w2s_col[:, None], 64, axis=1).astype(ml_dtypes.bfloat16)
    in_maps = []
    for k in range(NCORES):
        in_maps.append({
            "xat": np.ascontiguousarray(xa[k].T).astype(ml_dtypes.bfloat16),
            "w1t": w1t,
            "bcv": bcv,
            "w2s": w2s,
        })
    return in_maps


def kernel(xa, W1, b1, w2, b2):
    from concourse import bass_utils

    if "nc" not in _cached:
        _cached["nc"] = _build()
    nc = _cached["nc"]

    in_maps = _prep_in_maps(xa, W1, b1, w2, b2)
    res = bass_utils.run_bass_kernel_spmd(nc, in_maps, core_ids=list(range(NCORES)))
    out = np.stack([np.asarray(r["out"], dtype=np.float32) for r in res.results])
    return out


# revision 16
# speedup vs baseline: 1.0092x; 1.0092x over previous
"""Trainium2 Bass kernel for EdgeSelectionRL (gnn_message_passing).

Reference math (per batch b):
    a = xa @ Wa.T            (C, H)
    c = xa @ Wb.T            (C, H)
    logit[i, j] = sum_h w2[h] * relu(a[i, h] + c[j, h] + b1[h]) + b2
    out = sigmoid(logit)     (C, C)

Sharding: pure data-parallel over batch B=8 -> one batch element per core.

Structure (per core):
  * h-permutation pairing: h-indices paired (h1[p], h2[p]) with matching w2
    signs so alpha_p = w2[h2]/w2[h1] > 0; chunk-1 rows of W1/b1 are
    pre-scaled by alpha on the host, so BOTH h-chunk matmuls share ONE
    stationary w2[h1[p]] tile.
  * producers: relu(a_i + c_j + b1) tiles [128h, 256j] made on VectorE
    (tensor_scalar add+max from SBUF bf16, 2x mode, ~196ns) and ScalarE
    (activation Relu from PSUM, ~360ns), split ~360/152.
  * per 8-pair sweep TensorE accumulates 16 N=512 matmuls into 2 PSUM banks
    at 4 col-group positions; one ScalarE sigmoid per sweep; DMA pulls the
    16 valid rows out.
"""

import numpy as np

B, C, F, H = 8, 256, 128, 256
NCORES = 8
NPAIR = C // 2            # 128 i-pairs per core
PAIRS_PER_SWEEP = 8       # 2 banks x 4 col-groups
NSWEEP = 16               # 16 sweeps of 8 pairs
SIG_DEFER_AT = 5          # emit sweep s-1's sigmoid after this pair of sweep s
NS_UNITS = 146            # of 512 producer units on ScalarE (rest VectorE)

_cached = {}


def _unit_engines():
    """Pick NS_UNITS producer units for ScalarE, spread evenly over pairs and
    ordered so consecutive ScalarE ops strictly alternate the cTp chunk m
    (PSUM bank) — same-bank back-to-back reads cost +109ns each."""
    eng = ["V"] * 512
    last_m = 1   # next S-op should use m = 1 - last_m
    debt = 0
    scount = 0
    for q in range(128):
        want = (q + 1) * NS_UNITS // 128 - q * NS_UNITS // 128 + debt
        debt = 0
        if want >= 2 and last_m == 0:
            # emission within a pair is m0-then-m1; entering with last_m==0
            # would put m0 after m0 -> take one unit (m1) and defer the rest
            debt = want - 1
            want = 1
            ms = [1]
        elif want >= 2:
            want = min(want, 2)
            debt += max(0, want - 2)
            ms = [0, 1]
        elif want == 1:
            ms = [1 - last_m]
        else:
            ms = []
        for m in ms:
            hh = (scount // 2) % 2
            eng[4 * q + 2 * m + hh] = "S"
            last_m = m
            scount += 1
    return eng


def _build():
    import concourse.bass as bass
    import concourse.bacc as bacc
    import concourse.mybir as mybir
    from concourse import tile

    fp32 = mybir.dt.float32
    bf16 = mybir.dt.bfloat16
    Alu = mybir.AluOpType
    Act = mybir.ActivationFunctionType

    engines = _unit_engines()

    nc = bacc.Bacc(None, target_bir_lowering=False)

    xat_d = nc.dram_tensor("xat", [F, C], bf16, kind="ExternalInput")
    w1t_d = nc.dram_tensor("w1t", [2 * F, H], bf16, kind="ExternalInput")
    bcv_d = nc.dram_tensor("bcv", [128, 3], fp32, kind="ExternalInput")
    w2s_d = nc.dram_tensor("w2s", [128, 64], bf16, kind="ExternalInput")
    out_d = nc.dram_tensor("out", [C, C], fp32, kind="ExternalOutput")

    with tile.TileContext(nc) as tc:
        with (
            tc.tile_pool(name="const", bufs=1) as const_pool,
            tc.tile_pool(name="rtiles", bufs=24) as r_pool,
            tc.tile_pool(name="sig", bufs=4) as sig_pool,
            tc.tile_pool(name="psum", bufs=3, space=bass.MemorySpace.PSUM) as ps_pool,
            tc.tile_pool(name="psumc", bufs=1, space=bass.MemorySpace.PSUM) as psc_pool,
        ):
            # ---- load inputs (c-part of W1 first: cT gates VectorE ramp) ----
            xat = const_pool.tile([F, C], bf16, tag="xat")
            w1t = const_pool.tile([128, 2 * H], bf16, tag="w1t")  # [p, m2*H+h]
            bcv = const_pool.tile([128, 3], fp32, tag="bcv")      # b1 c0, c1, b2
            w2s = const_pool.tile([128, 64], bf16, tag="w2s")
            nc.sync.dma_start(xat[:], xat_d[:])
            nc.sync.dma_start(w1t[:, H:2 * H], w1t_d[128:256, :])   # Wb' part
            nc.sync.dma_start(w1t[:, 0:H], w1t_d[0:128, :])         # Wa' part
            nc.sync.dma_start(bcv[:], bcv_d[:])
            nc.sync.dma_start(w2s[:], w2s_d[:])
            w1t0 = w1t[:, 0:H]
            w1t1 = w1t[:, H:2 * H]
            b1p = bcv[:, 0:2]
            b2v = bcv[:, 2:3]

            # preload the sigmoid ACT table while DMAs are in flight
            warm = const_pool.tile([128, 1], fp32, tag="warm")
            nc.scalar.activation(
                warm[:], nc.const_aps.aps[(fp32, 0.0)], Act.Sigmoid,
            )

            # ---- setup ----
            aT = [const_pool.tile([128, C], fp32, tag=f"aT{m}", name=f"aT{m}")
                  for m in range(2)]
            aTb = [const_pool.tile([128, C], fp32, tag=f"aTb{m}", name=f"aTb{m}")
                   for m in range(2)]
            cT = [const_pool.tile([128, C], bf16, tag=f"cT{m}", name=f"cT{m}")
                  for m in range(2)]
            cTp = [psc_pool.tile([128, C], fp32, tag=f"cTp{m}", name=f"cTp{m}")
                   for m in range(2)]
            aps = ps_pool.tile([128, 1024], fp32, tag="ps")
            for m in range(2):
                # c chunk m -> cTp (PSUM, stays resident for ScalarE units)
                nc.tensor.matmul(
                    cTp[m][:], w1t1[:, m * 128:(m + 1) * 128],
                    xat[:], start=True, stop=True,
                )
                # cT (bf16, +b1) for VectorE units — on ScalarE, which is
                # idle during the ramp, so VectorE can start producing sooner
                nc.scalar.activation(
                    cT[m][:], cTp[m][:], Act.Identity, bias=b1p[:, m:m + 1],
                )
                nc.tensor.matmul(
                    aps[:, m * C:(m + 1) * C], w1t0[:, m * 128:(m + 1) * 128],
                    xat[:], start=True, stop=True,
                )
                nc.vector.tensor_copy(aT[m][:], aps[:, m * C:(m + 1) * C])
                nc.scalar.activation(
                    aTb[m][:], aps[:, m * C:(m + 1) * C], Act.Identity,
                    bias=b1p[:, m:m + 1],
                )

            # ---- main loop ----
            def _emit_sig(s, ps):
                sig = sig_pool.tile([128, 1024], fp32, tag="sig", name=f"sig{s}")
                nc.scalar.activation(sig[:], ps[:], Act.Sigmoid, bias=b2v[:, 0:1])
                # valid rows: partition 32*grp, free bank*512+hh*256 ->
                # out row i = 16*s + 8*bank + 2*grp + hh
                srcap = sig[0:128:32, :].rearrange("g (b e) -> g b e", b=2)
                dstap = out_d[16 * s:16 * s + 16, :].rearrange(
                    "(b g two) j -> g b (two j)", b=2, g=4)
                nc.sync.dma_start(dstap, srcap)

            def _emit_sig_bank(bk, ps):
                # tail sweep: per-bank sigmoid, rows 240+8*bk..247+8*bk
                sigb = sig_pool.tile([128, 512], fp32, tag="sig", name=f"sigb{bk}")
                nc.scalar.activation(sigb[:], ps[:, bk * 512:(bk + 1) * 512],
                                     Act.Sigmoid, bias=b2v[:, 0:1])
                dstb = out_d[240 + 8 * bk:248 + 8 * bk, :].rearrange(
                    "(g two) j -> g (two j)", g=4)
                nc.sync.dma_start(dstb, sigb[0:128:32, :])

            def _emit_unit(q, m, hh, dst):
                u = 4 * q + 2 * m + hh
                i = 2 * q + hh
                if engines[u] == "S":
                    nc.scalar.activation(
                        dst, cTp[m][:], Act.Relu,
                        bias=aTb[m][:, i:i + 1],
                    )
                else:
                    nc.vector.tensor_scalar(
                        dst, cT[m][:], aT[m][:, i:i + 1], 0.0,
                        Alu.add, Alu.max,
                    )

            pending = None
            for s in range(NSWEEP):
                ps = ps_pool.tile([128, 1024], fp32, tag="ps")
                for t in range(PAIRS_PER_SWEEP):
                    q = s * PAIRS_PER_SWEEP + t   # pair; i = 2q, 2q+1
                    bank = t // 4
                    grp = t % 4
                    rts = [r_pool.tile([128, 512], bf16, tag="r", name=f"r{q}_{m}")
                           for m in range(2)]
                    if t == SIG_DEFER_AT and pending is not None:
                        _emit_sig(*pending)
                        pending = None
                    for m in range(2):
                        for hh in range(2):
                            _emit_unit(q, m, hh, rts[m][:, hh * 256:(hh + 1) * 256])
                    po = ps[32 * grp:32 * grp + 32, bank * 512:(bank + 1) * 512]
                    nc.tensor.matmul(po, w2s[:, 0:32], rts[0][:],
                                     start=True, stop=False,
                                     tile_position=(0, 32 * grp))
                    nc.tensor.matmul(po, w2s[:, 32:64], rts[1][:],
                                     start=False, stop=True,
                                     tile_position=(0, 32 * grp))
                    if s == NSWEEP - 1 and t == 3:
                        _emit_sig_bank(0, ps)
                pending = (s, ps)
            _emit_sig_bank(1, pending[1])

    nc.compile()
    return nc


def _prep_in_maps(xa, W1, b1, w2, b2):
    import ml_dtypes

    xa = np.asarray(xa, dtype=np.float32)
    W1 = np.asarray(W1, dtype=np.float32)
    b1 = np.asarray(b1, dtype=np.float32).reshape(H)
    w2 = np.asarray(w2, dtype=np.float32).reshape(H)
    b2 = np.float32(np.asarray(b2).reshape(()))

    # pair h indices by matching w2 sign, sorted by |w2| so alpha ~ 1.
    # slot p gets (h1[p], h2[p]); chunk-1 rows are scaled by
    # alpha_p = w2[h2[p]] / w2[h1[p]] > 0, so both chunks share the
    # stationary w2[h1[p]].
    pos = np.where(w2 > 0)[0]
    neg = np.where(w2 <= 0)[0]
    pos = pos[np.argsort(-np.abs(w2[pos]), kind="stable")]
    neg = neg[np.argsort(-np.abs(w2[neg]), kind="stable")]
    assert len(pos) % 2 == 0 and len(neg) % 2 == 0, (len(pos), len(neg))
    h1 = np.concatenate([pos[0::2], neg[0::2]])
    h2 = np.concatenate([pos[1::2], neg[1::2]])
    alpha = w2[h2] / w2[h1]                      # > 0, ~1
    perm = np.concatenate([h1, h2])              # new h-order, chunk-major
    scale = np.concatenate([np.ones(128, np.float32), alpha.astype(np.float32)])

    W1n = W1[perm] * scale[:, None]              # (H, 2F) rows permuted+scaled
    b1n = b1[perm] * scale
    w2s_col = w2[h1].astype(np.float32)          # shared stationary column

    w1t = np.ascontiguousarray(W1n.T).astype(ml_dtypes.bfloat16)  # (2F, H)
    bcv = np.empty((128, 3), dtype=np.float32)
    bcv[:, 0:2] = b1n.reshape(2, 128).T
    bcv[:, 2] = b2
    w2s = np.repeat(w2s_col[:, None], 64, axis=1).astype(ml_dtypes.bfloat16)
    in_maps = []
    for k in range(NCORES):
        in_maps.append({
            "xat": np.ascontiguousarray(xa[k].T).astype(ml_dtypes.bfloat16),
            "w1t": w1t,
            "bcv": bcv,
            "w2s": w2s,
        })
    return in_maps


def kernel(xa, W1, b1, w2, b2):
    from concourse import bass_utils

    if "nc" not in _cached:
        _cached["nc"] = _build()
    nc = _cached["nc"]

    in_maps = _prep_in_maps(xa, W1, b1, w2, b2)
    res = bass_utils.run_bass_kernel_spmd(nc, in_maps, core_ids=list(range(NCORES)))
    out = np.stack([np.asarray(r["out"], dtype=np.float32) for r in res.results])
    return out
